# revision 17
# baseline (speedup 1.0000x reference)
"""Trainium2 Bass kernel for nn_EnokeeEncoder (segment_reduce).

Reference semantics:
    lhs = embed[input_ids]                      # only lhs[:, :32, :] is ever used
    m[b,j,x] = (pos[b,j,x] != -1) & (am[b,j] != 0)
    pooled = einsum('bml,bld->bmd', m, lhs[:, :32]) / 32
    x = LayerNorm(pooled) * gamma + beta
    out = (x @ w1) @ w2 + b2                    # [16, 64, 100000]

Device strategy (8 cores, SPMD, no collectives):
  - tensor-parallel over the entity vocab: core c owns w2[:, c*12500:(c+1)*12500]
  - every core redundantly computes hT = (LN(pooled) @ w1).T in [R=128, T=1024]
    layout, then the big projection runs with w2 chunks as the STATIONARY
    operand:  outT[e, t] = w2[:, e-chunk].T @ hT  so one weight load covers
    1024 moving columns.
  - outT goes to DRAM in bf16 (halves the dominant HBM write traffic); the
    host transposes + upcasts when unsharding.  w2 is pre-cast to bf16 on the
    host; hT is rounded to bf16; PSUM accumulates fp32 (rel-err ~3e-3 vs the
    2e-2 tolerance).
  - the pooled tensor itself is never materialized on device.  With the
    mask M [x, t] (block-diagonal, scaled by 1/L, built on the host) and
    E = embed rows [x, d]:
        yT   = (E @ w1g).T @ M          (w1g = gamma(.)w1, folded on host)
        mu   = (rowsum(E)/D).T @ M
        s2   = 1_x.T @ ((G @ M) (.) M), G = E @ E.T   (Gram, host-folded)
    so the device only contracts small [128 x 128] matrices against the mask
    (~6K PE columns total), instead of pooling the full [D, T] tensor.
  - LayerNorm is folded algebraically:  h = rs*y + (-rs*mu)*u + c  with
    u = gamma @ w1, c = beta @ w1 (host-folded), rs = 1/sqrt(var+eps).
"""

import sys

if '/opt/trn_rl_repo' not in sys.path:
    sys.path.insert(0, '/opt/trn_rl_repo')

import numpy as np
import ml_dtypes

import concourse.bass as bass
import concourse.mybir as mybir
import concourse.tile as tile
from concourse import bacc
from concourse.bass_utils import run_bass_kernel_spmd

# model dims (fixed by the problem)
B, S, M, L, D = 16, 512, 64, 32, 1024
V, R, E = 32000, 128, 100000
LN_EPS = 1e-5

N_CORES = 8
T = B * M              # 1024 mention-tokens
ES = E // N_CORES      # 12500 entity columns per core
ECH = 125              # entities per chunk (stationary M dim)
NEC = ES // ECH        # 100 chunks
QC = 2                 # chunks per output DMA
NG = NEC // QC         # 50 DMA groups
NW2 = 4                # w2 load chunks
W2CH = ES // NW2       # 3125 cols per w2 load

F32 = mybir.dt.float32
F32R = mybir.dt.float32r    # fp32 data, PE rounds (~tf32)
BF16 = mybir.dt.bfloat16
AF = mybir.AluOpType
ACTF = mybir.ActivationFunctionType


def build_nc(has_b2: bool):
    nc = bacc.Bacc("TRN2", target_bir_lowering=False, debug=False,
                   enable_asserts=False, num_devices=N_CORES)

    # ---- DRAM I/O (per-core) ----
    d_mblk = nc.dram_tensor("mblk", [128, 4 * 256], F32R, kind="ExternalInput").ap()
    d_gram = nc.dram_tensor("gram", [128, 4 * 128], F32R, kind="ExternalInput").ap()
    d_ew1 = nc.dram_tensor("ew1", [128, 4 * 128], F32R, kind="ExternalInput").ap()
    d_rsum = nc.dram_tensor("rsum", [128, 4], F32R, kind="ExternalInput").ap()
    d_cu = nc.dram_tensor("cu", [128, 2], F32, kind="ExternalInput").ap()
    d_ones = nc.dram_tensor("onesb", [128, 2], F32R, kind="ExternalInput").ap()
    d_onesr = nc.dram_tensor("onesr", [1, 128], F32R, kind="ExternalInput").ap()
    d_w2 = nc.dram_tensor("w2s", [R, ES], BF16, kind="ExternalInput").ap()
    d_b2 = nc.dram_tensor("b2s", [128, NEC], F32, kind="ExternalInput").ap()
    d_out = nc.dram_tensor("out", [ES, T], BF16, kind="ExternalOutput").ap()

    with tile.TileContext(nc) as tc:
        with (
            tc.tile_pool(name="persist", bufs=1) as pp,
            tc.tile_pool(name="pre", bufs=1) as pre,
        ):
            w2_sb = pp.tile([R, ES], BF16)
            hT_sb = pp.tile([R, T], BF16)

            # ---- input DMAs: small/critical first, w2 split across queues ----
            mblk_sb = pre.tile([128, T], F32R)
            nc.sync.dma_start(mblk_sb[:], d_mblk[:])
            gram_sb = pre.tile([128, 4, 128], F32R)
            nc.scalar.dma_start(gram_sb[:], d_gram[:])
            ew1_sb = pre.tile([128, 4, 128], F32R)
            nc.scalar.dma_start(ew1_sb[:], d_ew1[:])
            rsum_sb = pre.tile([128, 4], F32R)
            nc.sync.dma_start(rsum_sb[:], d_rsum[:])
            ones_sb = pre.tile([128, 2], F32R)
            nc.sync.dma_start(ones_sb[:], d_ones[:])
            onesr_sb = pre.tile([1, 128], F32R)
            nc.sync.dma_start(onesr_sb[:], d_onesr[:])
            cu_sb = pre.tile([128, 2], F32)
            nc.scalar.dma_start(cu_sb[:], d_cu[:])
            if has_b2:
                b2_sb = pre.tile([128, NEC], F32)
                nc.scalar.dma_start(b2_sb[:], d_b2[:])
            for wc in range(NW2):
                wsl = slice(wc * W2CH, (wc + 1) * W2CH)
                eng = nc.sync if wc % 2 == 0 else nc.scalar
                eng.dma_start(w2_sb[:, wsl], d_w2[:, wsl])

            # ---- prefix: stats + classifier input, no pooled tensor ----
            with tc.tile_pool(name="preps", bufs=1, space="PSUM") as prps:
                yT_ps = prps.tile([128, T], F32)
                wps_cm = tc.tile_pool(name="wps", bufs=1, space="PSUM")
                wps = wps_cm.__enter__()
                W_ps = wps.tile([128, T], F32)
                s2_ps = wps.tile([1, T], F32)
                mu_ps = wps.tile([1, T], F32)

                # W = G @ M and yT = (E w1g).T @ M and mu = rsum.T @ M
                for g in range(4):
                    gsl = slice(g * 256, (g + 1) * 256)
                    nc.tensor.matmul(out=W_ps[:, gsl], lhsT=gram_sb[:, g, :],
                                     rhs=mblk_sb[:, g * 256:(g + 1) * 256], start=True, stop=True,
                                     skip_group_check=True)
                    nc.tensor.matmul(out=yT_ps[:, gsl], lhsT=ew1_sb[:, g, :],
                                     rhs=mblk_sb[:, g * 256:(g + 1) * 256], start=True, stop=True,
                                     skip_group_check=True)
                    nc.tensor.matmul(out=mu_ps[:, gsl], lhsT=rsum_sb[:, g:g + 1],
                                     rhs=mblk_sb[:, g * 256:(g + 1) * 256], start=True, stop=True,
                                     skip_group_check=True)
                # WM = W (.) M  -> s2 = 1.T @ WM
                wm_sb = pre.tile([128, T], F32R)
                mu_sb = pre.tile([1, T], F32R)
                e2_sb = pre.tile([1, T], F32R)
                for nch in range(2):
                    sl = slice(nch * 512, (nch + 1) * 512)
                    nc.vector.tensor_tensor(
                        wm_sb[:, sl], W_ps[:, sl],
                        mblk_sb[:, sl], op=AF.mult)
                    nc.tensor.matmul(out=s2_ps[:, sl], lhsT=ones_sb[:, 0:1],
                                     rhs=wm_sb[:, sl], start=True, stop=True,
                                     skip_group_check=True)
                nc.vector.tensor_copy(mu_sb[:], mu_ps[:])
                nc.vector.tensor_scalar(e2_sb[:], s2_ps[:], 1.0 / D, None,
                                        op0=AF.mult)
                wps_cm.__exit__(None, None, None)

                # broadcast mu, E[x^2] across partitions; LN fold -> hT bf16
                with tc.tile_pool(name="bcps", bufs=1, space="PSUM") as bps:
                    mub_ps = bps.tile([128, T], F32)
                    e2b_ps = bps.tile([128, T], F32)
                    for nch in range(2):
                        sl = slice(nch * 512, (nch + 1) * 512)
                        nc.tensor.matmul(out=mub_ps[:, sl], lhsT=onesr_sb[:],
                                         rhs=mu_sb[:, sl], start=True, stop=True,
                                         skip_group_check=True)
                        nc.tensor.matmul(out=e2b_ps[:, sl], lhsT=onesr_sb[:],
                                         rhs=e2_sb[:, sl], start=True, stop=True,
                                         skip_group_check=True)

                    musq_sb = pre.tile([128, T], F32)
                    vare_sb = pre.tile([128, T], F32)
                    rs_sb = pre.tile([128, T], F32)
                    nmurs_sb = pre.tile([128, T], F32)
                    t1_sb = pre.tile([128, T], F32)
                    t2_sb = pre.tile([128, T], F32)
                    for nch in range(2):
                        sl = slice(nch * 512, (nch + 1) * 512)
                        nc.scalar.square(musq_sb[:, sl], mub_ps[:, sl])
                        nc.vector.scalar_tensor_tensor(vare_sb[:, sl],
                                                       in0=e2b_ps[:, sl],
                                                       scalar=LN_EPS,
                                                       in1=musq_sb[:, sl],
                                                       op0=AF.add,
                                                       op1=AF.subtract)
                        nc.scalar.activation(rs_sb[:, sl], vare_sb[:, sl],
                                             ACTF.Abs_reciprocal_sqrt)
                        nc.vector.scalar_tensor_tensor(nmurs_sb[:, sl],
                                                       in0=mub_ps[:, sl],
                                                       scalar=-1.0,
                                                       in1=rs_sb[:, sl],
                                                       op0=AF.mult, op1=AF.mult)
                        nc.vector.tensor_tensor(t1_sb[:, sl], yT_ps[:, sl],
                                                rs_sb[:, sl], op=AF.mult)
                        nc.vector.scalar_tensor_tensor(t2_sb[:, sl],
                                                       in0=nmurs_sb[:, sl],
                                                       scalar=cu_sb[:, 1:2],
                                                       in1=t1_sb[:, sl],
                                                       op0=AF.mult, op1=AF.add)
                        nc.vector.tensor_scalar(hT_sb[:, sl], t2_sb[:, sl],
                                                cu_sb[:, 0:1], None, op0=AF.add)

            # ---- main: outT[e-chunk, t] = w2[:, e-chunk].T @ hT ----
            # 2 chunks of 125 entities share one output DMA (250 rows);
            # dest AP maps SBUF (p, k) -> DRAM row 250*grp + 125*k + p.
            d_out_r = d_out.rearrange("(g k p) t -> g p k t", g=NG, k=QC, p=ECH)
            # Software-pipelined emission: the tile scheduler gates each
            # instruction on per-engine program-order progress, so emission
            # order IS the schedule.  Emit matmuls for chunk i, evacuations
            # for chunk i-2, and the group DMA for chunks i-5/i-4 so every
            # wait is already satisfied when the instruction issues.
            EV_LAG = 2
            DMA_LAG = 4
            with tc.tile_pool(name="mainps", bufs=8, space="PSUM") as mps2, \
                 tc.tile_pool(name="outp", bufs=8) as op:
                mm_tiles = {}
                o2_tiles = {}
                for i in range(NEC + DMA_LAG + 1):
                    if i < NEC:
                        esl = slice(i * ECH, (i + 1) * ECH)
                        for h in range(2):
                            sl = slice(h * 512, (h + 1) * 512)
                            mm = mps2.tile([128, 512], F32, tag="mm",
                                           name=f"mm{i}_{h}")
                            mm_tiles[(i, h)] = mm
                            nc.tensor.matmul(out=mm[0:ECH, :],
                                             lhsT=w2_sb[:, esl],
                                             rhs=hT_sb[:, sl],
                                             start=True, stop=True)
                    j = i - EV_LAG
                    if 0 <= j < NEC:
                        g, k = divmod(j, QC)
                        if k == 0:
                            o2_tiles[g] = op.tile([128, QC, T], BF16, tag="o",
                                                  name=f"o2g{g}")
                        o2 = o2_tiles[g]
                        m0 = mm_tiles.pop((j, 0))
                        m1 = mm_tiles.pop((j, 1))
                        if has_b2:
                            nc.vector.tensor_scalar(o2[0:ECH, k, 0:512],
                                                    m0[0:ECH, :],
                                                    b2_sb[0:ECH, j:j + 1],
                                                    None, op0=AF.add)
                            nc.scalar.add(o2[0:ECH, k, 512:1024],
                                          m1[0:ECH, :],
                                          b2_sb[0:ECH, j:j + 1])
                        else:
                            nc.vector.tensor_copy(o2[0:ECH, k, 0:512],
                                                  m0[0:ECH, :])
                            nc.scalar.copy(o2[0:ECH, k, 512:1024],
                                           m1[0:ECH, :])
                    kk = i - DMA_LAG
                    if 0 <= kk < NEC and kk % QC == QC - 1:
                        g = kk // QC
                        nc.sync.dma_start(d_out_r[g],
                                          o2_tiles.pop(g)[0:ECH, :, :])

    nc.finalize()
    return nc


_NC_CACHE = {}


def _get_nc(has_b2: bool):
    if has_b2 not in _NC_CACHE:
        _NC_CACHE[has_b2] = build_nc(has_b2)
    return _NC_CACHE[has_b2]


def prep_core_inputs(inputs):
    """Host-side sharding/layout prep. Returns (shared_map, per_core_w2, per_core_b2, has_b2)."""
    ids = np.asarray(inputs["input_ids"]).astype(np.int32)[:, :L]      # [16, 32]
    pos = np.asarray(inputs["entity_position_ids"]).astype(np.int32)   # [16, 64, 32]
    am = np.asarray(inputs["entity_attention_mask"]).astype(np.int32)  # [16, 64]
    embed = np.ascontiguousarray(np.asarray(inputs["embed"], dtype=np.float32))
    gamma = np.asarray(inputs["ln_gamma"], dtype=np.float32)
    beta = np.asarray(inputs["ln_beta"], dtype=np.float32)
    w1 = np.ascontiguousarray(np.asarray(inputs["w1"], dtype=np.float32))
    w2 = np.asarray(inputs["w2"], dtype=np.float32)
    b2 = np.asarray(inputs["b2"], dtype=np.float32)

    # E_g[p=32b+x, g, :] = embed[ids[4g+b, x]]  (batch 4g+b on k-block b)
    emb_idx = ids.reshape(4, 4, L).transpose(1, 2, 0).reshape(128, 4)
    emb_g = embed[emb_idx]                                  # [128, 4, D]
    rsum = np.ascontiguousarray(
        emb_g.sum(axis=2, dtype=np.float64).astype(np.float32) / D)   # [128, 4]

    # block-diagonal mask, scaled by 1/L:
    #   mblk[32k+x, g, 64k+j] = m[4g+k, j, x] / L
    mask = (((pos != -1) & (am[:, :, None] != 0)).astype(np.float32) / L)  # [b,j,x]
    mblk = np.zeros((128, 4, 256), np.float32)
    for g in range(4):
        for k in range(4):
            mblk[32 * k:32 * (k + 1), g, 64 * k:64 * (k + 1)] = \
                mask[4 * g + k].T
    mblk = np.ascontiguousarray(mblk.reshape(128, 4 * 256))

    # host-folded small matrices (mask-independent):
    #   gram[:, g, :] = E_g @ E_g.T ; ew1[:, g, :] = E_g @ (gamma (.) w1)
    w1g = gamma[:, None] * w1                               # [D, R]
    eg64 = emb_g.astype(np.float64)
    gram = np.einsum('pgd,qgd->pgq', eg64, eg64).astype(np.float32)
    ew1 = np.einsum('pgd,dr->pgr', eg64, w1g.astype(np.float64)).astype(np.float32)
    cu = np.ascontiguousarray(
        np.stack([beta @ w1, gamma @ w1], axis=-1))         # [R, 2]

    shared = {
        "mblk": mblk,
        "gram": np.ascontiguousarray(gram.reshape(128, 4 * 128)),
        "ew1": np.ascontiguousarray(ew1.reshape(128, 4 * 128)),
        "rsum": rsum,
        "cu": cu,
        "onesb": np.ones((128, 2), np.float32),
        "onesr": np.ones((1, 128), np.float32),
    }
    w2s = [np.ascontiguousarray(w2[:, c * ES:(c + 1) * ES]).astype(ml_dtypes.bfloat16)
           for c in range(N_CORES)]
    b2s = [np.ascontiguousarray(
               np.pad(b2[c * ES:(c + 1) * ES].reshape(NEC, ECH).T,
                      ((0, 128 - ECH), (0, 0))))
           for c in range(N_CORES)]
    has_b2 = bool(np.any(b2 != 0.0))
    return shared, w2s, b2s, has_b2


def kernel(**inputs) -> np.ndarray:
    shared, w2s, b2s, has_b2 = prep_core_inputs(inputs)
    nc = _get_nc(has_b2)
    in_maps = [dict(shared, w2s=w2s[c], b2s=b2s[c]) for c in range(N_CORES)]
    res = run_bass_kernel_spmd(nc, in_maps, list(range(N_CORES)))
    full = np.empty((T, E), np.float32)
    for c in range(N_CORES):
        outT = np.asarray(res.results[c]["out"])               # [ES, T] bf16
        full[:, c * ES:(c + 1) * ES] = outT.T.astype(np.float32)
    return np.ascontiguousarray(full.reshape(B, M, E))


# revision 18
# speedup vs baseline: 1.0767x; 1.0767x over previous
"""Trainium2 Bass kernel for nn_EnokeeEncoder (segment_reduce).

Reference semantics:
    lhs = embed[input_ids]                      # only lhs[:, :32, :] is ever used
    m[b,j,x] = (pos[b,j,x] != -1) & (am[b,j] != 0)
    pooled = einsum('bml,bld->bmd', m, lhs[:, :32]) / 32
    x = LayerNorm(pooled) * gamma + beta
    out = (x @ w1) @ w2 + b2                    # [16, 64, 100000]

Device strategy (8 cores, SPMD, no collectives):
  - tensor-parallel over the entity vocab: core c owns w2[:, c*12500:(c+1)*12500]
  - every core redundantly computes hT = (LN(pooled) @ w1).T in [R=128, T=1024]
    layout, then the big projection runs with w2 chunks as the STATIONARY
    operand:  outT[e, t] = w2[:, e-chunk].T @ hT  so one weight load covers
    1024 moving columns.
  - outT goes to DRAM in bf16 (halves the dominant HBM write traffic); the
    host transposes + upcasts when unsharding.  w2 is pre-cast to bf16 on the
    host; hT is rounded to bf16; PSUM accumulates fp32 (rel-err ~3e-3 vs the
    2e-2 tolerance).
  - the pooled tensor itself is never materialized on device.  With the
    mask M [x, t] (block-diagonal, scaled by 1/L, built on the host) and
    E = embed rows [x, d]:
        yT   = (E @ w1g).T @ M          (w1g = gamma(.)w1, folded on host)
        mu   = (rowsum(E)/D).T @ M
        s2   = 1_x.T @ ((G @ M) (.) M), G = E @ E.T   (Gram, host-folded)
    so the device only contracts small [128 x 128] matrices against the mask
    (~6K PE columns total), instead of pooling the full [D, T] tensor.
  - LayerNorm is folded algebraically:  h = rs*y + (-rs*mu)*u + c  with
    u = gamma @ w1, c = beta @ w1 (host-folded), rs = 1/sqrt(var+eps).
"""

import sys

if '/opt/trn_rl_repo' not in sys.path:
    sys.path.insert(0, '/opt/trn_rl_repo')

import numpy as np
import ml_dtypes

import concourse.bass as bass
import concourse.mybir as mybir
import concourse.tile as tile
from concourse import bacc
from concourse.bass_utils import run_bass_kernel_spmd

# model dims (fixed by the problem)
B, S, M, L, D = 16, 512, 64, 32, 1024
V, R, E = 32000, 128, 100000
LN_EPS = 1e-5

N_CORES = 8
T = B * M              # 1024 mention-tokens
ES = E // N_CORES      # 12500 entity columns per core
ECH = 125              # entities per chunk (stationary M dim)
NEC = ES // ECH        # 100 chunks
QC = 4                 # chunks per output DMA
NG = NEC // QC         # 25 DMA groups
NW2 = 4                # w2 load chunks
W2CH = ES // NW2       # 3125 cols per w2 load

F32 = mybir.dt.float32
F32R = mybir.dt.float32r    # fp32 data, PE rounds (~tf32)
BF16 = mybir.dt.bfloat16
AF = mybir.AluOpType
ACTF = mybir.ActivationFunctionType


def build_nc(has_b2: bool):
    nc = bacc.Bacc("TRN2", target_bir_lowering=False, debug=False,
                   enable_asserts=False, num_devices=N_CORES)

    # ---- DRAM I/O (per-core) ----
    d_mblk = nc.dram_tensor("mblk", [128, 4 * 256], F32R, kind="ExternalInput").ap()
    d_gram = nc.dram_tensor("gram", [128, 4 * 128], F32R, kind="ExternalInput").ap()
    d_ew1 = nc.dram_tensor("ew1", [128, 4 * 128], F32R, kind="ExternalInput").ap()
    d_rsum = nc.dram_tensor("rsum", [128, 4], F32R, kind="ExternalInput").ap()
    d_cu = nc.dram_tensor("cu", [128, 2], F32, kind="ExternalInput").ap()
    d_ones = nc.dram_tensor("onesb", [128, 2], F32R, kind="ExternalInput").ap()
    d_onesr = nc.dram_tensor("onesr", [1, 128], F32R, kind="ExternalInput").ap()
    d_w2 = nc.dram_tensor("w2s", [R, ES], BF16, kind="ExternalInput").ap()
    d_b2 = nc.dram_tensor("b2s", [128, NEC], F32, kind="ExternalInput").ap()
    d_out = nc.dram_tensor("out", [ES, T], BF16, kind="ExternalOutput").ap()

    with tile.TileContext(nc) as tc:
        with (
            tc.tile_pool(name="persist", bufs=1) as pp,
            tc.tile_pool(name="pre", bufs=1) as pre,
        ):
            w2_sb = pp.tile([R, ES], BF16)
            hT_sb = pp.tile([R, T], BF16)

            # ---- input DMAs: small/critical first, w2 split across queues ----
            mblk_sb = pre.tile([128, T], F32R)
            nc.sync.dma_start(mblk_sb[:], d_mblk[:])
            gram_sb = pre.tile([128, 4, 128], F32R)
            nc.scalar.dma_start(gram_sb[:], d_gram[:])
            ew1_sb = pre.tile([128, 4, 128], F32R)
            nc.scalar.dma_start(ew1_sb[:], d_ew1[:])
            rsum_sb = pre.tile([128, 4], F32R)
            nc.sync.dma_start(rsum_sb[:], d_rsum[:])
            ones_sb = pre.tile([128, 2], F32R)
            nc.sync.dma_start(ones_sb[:], d_ones[:])
            onesr_sb = pre.tile([1, 128], F32R)
            nc.sync.dma_start(onesr_sb[:], d_onesr[:])
            cu_sb = pre.tile([128, 2], F32)
            nc.scalar.dma_start(cu_sb[:], d_cu[:])
            if has_b2:
                b2_sb = pre.tile([128, NEC], F32)
                nc.scalar.dma_start(b2_sb[:], d_b2[:])
            for wc in range(NW2):
                wsl = slice(wc * W2CH, (wc + 1) * W2CH)
                eng = nc.sync if wc % 2 == 0 else nc.scalar
                eng.dma_start(w2_sb[:, wsl], d_w2[:, wsl])

            # ---- prefix: stats + classifier input, no pooled tensor ----
            with tc.tile_pool(name="preps", bufs=1, space="PSUM") as prps:
                yT_ps = prps.tile([128, T], F32)
                wps_cm = tc.tile_pool(name="wps", bufs=1, space="PSUM")
                wps = wps_cm.__enter__()
                W_ps = wps.tile([128, T], F32)
                s2_ps = wps.tile([1, T], F32)
                mu_ps = wps.tile([1, T], F32)

                # W = G @ M and yT = (E w1g).T @ M and mu = rsum.T @ M
                for g in range(4):
                    gsl = slice(g * 256, (g + 1) * 256)
                    nc.tensor.matmul(out=W_ps[:, gsl], lhsT=gram_sb[:, g, :],
                                     rhs=mblk_sb[:, g * 256:(g + 1) * 256], start=True, stop=True,
                                     skip_group_check=True)
                    nc.tensor.matmul(out=yT_ps[:, gsl], lhsT=ew1_sb[:, g, :],
                                     rhs=mblk_sb[:, g * 256:(g + 1) * 256], start=True, stop=True,
                                     skip_group_check=True)
                    nc.tensor.matmul(out=mu_ps[:, gsl], lhsT=rsum_sb[:, g:g + 1],
                                     rhs=mblk_sb[:, g * 256:(g + 1) * 256], start=True, stop=True,
                                     skip_group_check=True)
                # WM = W (.) M  -> s2 = 1.T @ WM
                wm_sb = pre.tile([128, T], F32R)
                mu_sb = pre.tile([1, T], F32R)
                e2_sb = pre.tile([1, T], F32R)
                for nch in range(2):
                    sl = slice(nch * 512, (nch + 1) * 512)
                    nc.vector.tensor_tensor(
                        wm_sb[:, sl], W_ps[:, sl],
                        mblk_sb[:, sl], op=AF.mult)
                    nc.tensor.matmul(out=s2_ps[:, sl], lhsT=ones_sb[:, 0:1],
                                     rhs=wm_sb[:, sl], start=True, stop=True,
                                     skip_group_check=True)
                nc.vector.tensor_copy(mu_sb[:], mu_ps[:])
                nc.vector.tensor_scalar(e2_sb[:], s2_ps[:], 1.0 / D, None,
                                        op0=AF.mult)
                wps_cm.__exit__(None, None, None)

                # broadcast mu, E[x^2] across partitions; LN fold -> hT bf16
                with tc.tile_pool(name="bcps", bufs=1, space="PSUM") as bps:
                    mub_ps = bps.tile([128, T], F32)
                    e2b_ps = bps.tile([128, T], F32)
                    for nch in range(2):
                        sl = slice(nch * 512, (nch + 1) * 512)
                        nc.tensor.matmul(out=mub_ps[:, sl], lhsT=onesr_sb[:],
                                         rhs=mu_sb[:, sl], start=True, stop=True,
                                         skip_group_check=True)
                        nc.tensor.matmul(out=e2b_ps[:, sl], lhsT=onesr_sb[:],
                                         rhs=e2_sb[:, sl], start=True, stop=True,
                                         skip_group_check=True)

                    musq_sb = pre.tile([128, T], F32)
                    vare_sb = pre.tile([128, T], F32)
                    rs_sb = pre.tile([128, T], F32)
                    nmurs_sb = pre.tile([128, T], F32)
                    t1_sb = pre.tile([128, T], F32)
                    t2_sb = pre.tile([128, T], F32)
                    for nch in range(2):
                        sl = slice(nch * 512, (nch + 1) * 512)
                        nc.scalar.square(musq_sb[:, sl], mub_ps[:, sl])
                        nc.vector.scalar_tensor_tensor(vare_sb[:, sl],
                                                       in0=e2b_ps[:, sl],
                                                       scalar=LN_EPS,
                                                       in1=musq_sb[:, sl],
                                                       op0=AF.add,
                                                       op1=AF.subtract)
                        nc.scalar.activation(rs_sb[:, sl], vare_sb[:, sl],
                                             ACTF.Abs_reciprocal_sqrt)
                        nc.vector.scalar_tensor_tensor(nmurs_sb[:, sl],
                                                       in0=mub_ps[:, sl],
                                                       scalar=-1.0,
                                                       in1=rs_sb[:, sl],
                                                       op0=AF.mult, op1=AF.mult)
                        nc.vector.tensor_tensor(t1_sb[:, sl], yT_ps[:, sl],
                                                rs_sb[:, sl], op=AF.mult)
                        nc.vector.scalar_tensor_tensor(t2_sb[:, sl],
                                                       in0=nmurs_sb[:, sl],
                                                       scalar=cu_sb[:, 1:2],
                                                       in1=t1_sb[:, sl],
                                                       op0=AF.mult, op1=AF.add)
                        nc.vector.tensor_scalar(hT_sb[:, sl], t2_sb[:, sl],
                                                cu_sb[:, 0:1], None, op0=AF.add)

            # ---- main: outT[e-chunk, t] = w2[:, e-chunk].T @ hT ----
            # 2 chunks of 125 entities share one output DMA (250 rows);
            # dest AP maps SBUF (p, k) -> DRAM row 250*grp + 125*k + p.
            d_out_r = d_out.rearrange("(g k p) t -> g p k t", g=NG, k=QC, p=ECH)
            # Software-pipelined emission with TWO DMA queues (sync/scalar
            # alternating per group) so output transfers overlap; a single
            # queue serializes end-to-end at ~217GB/s.  Evacuations run one
            # chunk per op (V/S alternating), two steps behind the matmuls.
            EV_LAG = 2
            DMA_LAG = 4
            with tc.tile_pool(name="mainps", bufs=4, space="PSUM") as mps2, \
                 tc.tile_pool(name="outp", bufs=4) as op:
                mm_tiles = {}
                o4_tiles = {}
                for i in range(NEC + DMA_LAG + 1):
                    if i < NEC:
                        esl = slice(i * ECH, (i + 1) * ECH)
                        mm = mps2.tile([128, T], F32, tag="mm", name=f"mm{i}")
                        mm_tiles[i] = mm
                        for h in range(2):
                            sl = slice(h * 512, (h + 1) * 512)
                            nc.tensor.matmul(out=mm[0:ECH, sl],
                                             lhsT=w2_sb[:, esl],
                                             rhs=hT_sb[:, sl],
                                             start=True, stop=True)
                    j = i - EV_LAG
                    if 0 <= j < NEC:
                        g, k = divmod(j, QC)
                        if k == 0:
                            o4_tiles[g] = op.tile([128, QC, T], BF16, tag="o",
                                                  name=f"o4g{g}")
                        o4 = o4_tiles[g]
                        mm = mm_tiles.pop(j)
                        if has_b2:
                            if j % 2 == 0:
                                nc.vector.tensor_scalar(o4[0:ECH, k, :],
                                                        mm[0:ECH, :],
                                                        b2_sb[0:ECH, j:j + 1],
                                                        None, op0=AF.add)
                            else:
                                nc.scalar.add(o4[0:ECH, k, :], mm[0:ECH, :],
                                              b2_sb[0:ECH, j:j + 1])
                        elif j % 2 == 0:
                            nc.vector.tensor_copy(o4[0:ECH, k, :], mm[0:ECH, :])
                        else:
                            nc.scalar.copy(o4[0:ECH, k, :], mm[0:ECH, :])
                    kk = i - DMA_LAG
                    if 0 <= kk < NEC and kk % QC == QC - 1:
                        g = kk // QC
                        dma_eng = nc.sync if g % 2 == 0 else nc.scalar
                        dma_eng.dma_start(d_out_r[g],
                                          o4_tiles.pop(g)[0:ECH, :, :])

    nc.finalize()
    return nc


_NC_CACHE = {}


def _get_nc(has_b2: bool):
    if has_b2 not in _NC_CACHE:
        _NC_CACHE[has_b2] = build_nc(has_b2)
    return _NC_CACHE[has_b2]


def prep_core_inputs(inputs):
    """Host-side sharding/layout prep. Returns (shared_map, per_core_w2, per_core_b2, has_b2)."""
    ids = np.asarray(inputs["input_ids"]).astype(np.int32)[:, :L]      # [16, 32]
    pos = np.asarray(inputs["entity_position_ids"]).astype(np.int32)   # [16, 64, 32]
    am = np.asarray(inputs["entity_attention_mask"]).astype(np.int32)  # [16, 64]
    embed = np.ascontiguousarray(np.asarray(inputs["embed"], dtype=np.float32))
    gamma = np.asarray(inputs["ln_gamma"], dtype=np.float32)
    beta = np.asarray(inputs["ln_beta"], dtype=np.float32)
    w1 = np.ascontiguousarray(np.asarray(inputs["w1"], dtype=np.float32))
    w2 = np.asarray(inputs["w2"], dtype=np.float32)
    b2 = np.asarray(inputs["b2"], dtype=np.float32)

    # E_g[p=32b+x, g, :] = embed[ids[4g+b, x]]  (batch 4g+b on k-block b)
    emb_idx = ids.reshape(4, 4, L).transpose(1, 2, 0).reshape(128, 4)
    emb_g = embed[emb_idx]                                  # [128, 4, D]
    rsum = np.ascontiguousarray(
        emb_g.sum(axis=2, dtype=np.float64).astype(np.float32) / D)   # [128, 4]

    # block-diagonal mask, scaled by 1/L:
    #   mblk[32k+x, g, 64k+j] = m[4g+k, j, x] / L
    mask = (((pos != -1) & (am[:, :, None] != 0)).astype(np.float32) / L)  # [b,j,x]
    mblk = np.zeros((128, 4, 256), np.float32)
    for g in range(4):
        for k in range(4):
            mblk[32 * k:32 * (k + 1), g, 64 * k:64 * (k + 1)] = \
                mask[4 * g + k].T
    mblk = np.ascontiguousarray(mblk.reshape(128, 4 * 256))

    # host-folded small matrices (mask-independent):
    #   gram[:, g, :] = E_g @ E_g.T ; ew1[:, g, :] = E_g @ (gamma (.) w1)
    w1g = gamma[:, None] * w1                               # [D, R]
    eg64 = emb_g.astype(np.float64)
    gram = np.einsum('pgd,qgd->pgq', eg64, eg64).astype(np.float32)
    ew1 = np.einsum('pgd,dr->pgr', eg64, w1g.astype(np.float64)).astype(np.float32)
    cu = np.ascontiguousarray(
        np.stack([beta @ w1, gamma @ w1], axis=-1))         # [R, 2]

    shared = {
        "mblk": mblk,
        "gram": np.ascontiguousarray(gram.reshape(128, 4 * 128)),
        "ew1": np.ascontiguousarray(ew1.reshape(128, 4 * 128)),
        "rsum": rsum,
        "cu": cu,
        "onesb": np.ones((128, 2), np.float32),
        "onesr": np.ones((1, 128), np.float32),
    }
    w2s = [np.ascontiguousarray(w2[:, c * ES:(c + 1) * ES]).astype(ml_dtypes.bfloat16)
           for c in range(N_CORES)]
    b2s = [np.ascontiguousarray(
               np.pad(b2[c * ES:(c + 1) * ES].reshape(NEC, ECH).T,
                      ((0, 128 - ECH), (0, 0))))
           for c in range(N_CORES)]
    has_b2 = bool(np.any(b2 != 0.0))
    return shared, w2s, b2s, has_b2


def kernel(**inputs) -> np.ndarray:
    shared, w2s, b2s, has_b2 = prep_core_inputs(inputs)
    nc = _get_nc(has_b2)
    in_maps = [dict(shared, w2s=w2s[c], b2s=b2s[c]) for c in range(N_CORES)]
    res = run_bass_kernel_spmd(nc, in_maps, list(range(N_CORES)))
    full = np.empty((T, E), np.float32)
    for c in range(N_CORES):
        outT = np.asarray(res.results[c]["out"])               # [ES, T] bf16
        full[:, c * ES:(c + 1) * ES] = outT.T.astype(np.float32)
    return np.ascontiguousarray(full.reshape(B, M, E))
